# revision 1
# baseline (speedup 1.0000x reference)
"""AstroGCNLayer Trainium2 kernel (8 NeuronCores, SPMD).

Strategy: edges are bucketed on host by destination core (row // 6250) so no
cross-core reduction is needed. Within a core, edges are sorted by
(dest-block, edge-type) and padded into 128-edge chunks. On device, per
chunk: dma_gather of x[col] (bf16), edge-MLP first layer as a matmul, a
destination one-hot built with a DVE is_equal, and two "segment matmuls"
(payload^T @ onehot) accumulating per-block type-sums S_t and MLP-sums P in
PSUM. Per 128-node block the remaining algebra collapses to seven small
matmuls (W_t, W_e2, W_self, per-type-count x bias), then +b_self, a PE
transpose, LayerNorm and ReLU.

The matmuls-commute-with-segment-sum identity removes all per-edge type
matmuls and the W_e2 matmul; dma_scatter_add is avoided entirely (its CCE
read-modify-write loses updates for duplicate indices - HW-verified).

dma_gather indices are int16 but N=50000: the gather base is x[32768:] and
indices are stored as col-32768; negative values address below the base
(HW-verified exact).
"""
import contextlib
import ctypes
import os
import sys

import numpy as np
import ml_dtypes

import concourse.bass as bass
import concourse.mybir as mybir
from concourse import bacc, tile

bf16 = ml_dtypes.bfloat16

N, E, D, T, NC = 50000, 500000, 128, 4, 8
NLOC = N // NC          # 6250 dest nodes per core
NBLK = (NLOC + 127) // 128   # 49 blocks (48 full + 106)
NCELL = NBLK * T
GCH = 8                 # chunks per gather call (dma_gather max 1024 idxs)
EFG = 32                # chunks per ef tile
EPS = 1e-5
IDX_BASE = 32768

_CACHE = {}
_PROFILE_DIR = None     # test.py sets this to capture an NTFF profile


def _plan(cnt_max):
    """Shared chunk plan from per-cell max counts (identical across cores)."""
    chunks_cell = np.maximum(1, -(-cnt_max // 128)).astype(np.int64)
    cell_chunk_start = np.concatenate([[0], np.cumsum(chunks_cell)])
    nchunks = int(cell_chunk_start[-1])
    plan = []
    for cell in range(NCELL):
        b, t = cell // T, cell % T
        for k in range(int(cell_chunk_start[cell]), int(cell_chunk_start[cell + 1])):
            plan.append(
                dict(
                    b=b,
                    t=t,
                    s_start=(k == cell_chunk_start[cell]),
                    s_stop=(k == cell_chunk_start[cell + 1] - 1),
                    p_start=(k == cell_chunk_start[b * T]),
                    p_stop=(k == cell_chunk_start[(b + 1) * T] - 1),
                    last_of_block=(k == cell_chunk_start[(b + 1) * T] - 1),
                )
            )
    return plan, nchunks, cell_chunk_start


def _build(nc_graph_key, plan, nchunks, ln_trivial):
    S = nchunks * 128
    nc = bacc.Bacc(None, target_bir_lowering=False, num_swdge_queues=1)
    f32, b16, i16 = mybir.dt.float32, mybir.dt.bfloat16, mybir.dt.int16

    xt_ext = nc.declare_dram_parameter("xt", [N, D], b16, isOutput=False)
    eft_ext = nc.declare_dram_parameter("eft", [D, S], b16, isOutput=False)
    gidx_ext = nc.declare_dram_parameter("gidx", [128, S // 16], i16, isOutput=False)
    dloc_ext = nc.declare_dram_parameter("dloc", [128, nchunks], f32, isOutput=False)
    cnts_ext = nc.declare_dram_parameter("cnts", [T, NLOC], b16, isOutput=False)
    xloc_ext = nc.declare_dram_parameter("xloc", [D, NLOC], b16, isOutput=False)
    wts_ext = nc.declare_dram_parameter("wts", [D, T * D], b16, isOutput=False)
    we1_ext = nc.declare_dram_parameter("we1", [D, D], b16, isOutput=False)
    we2_ext = nc.declare_dram_parameter("we2", [D, D], b16, isOutput=False)
    wself_ext = nc.declare_dram_parameter("wself", [D, D], b16, isOutput=False)
    bp_ext = nc.declare_dram_parameter("bp", [T, D], b16, isOutput=False)
    be1_ext = nc.declare_dram_parameter("be1", [1, D], b16, isOutput=False)
    bself_ext = nc.declare_dram_parameter("bself", [D, 1], f32, isOutput=False)
    lng_ext = nc.declare_dram_parameter("lng", [D, D], f32, isOutput=False)
    lnb_ext = nc.declare_dram_parameter("lnb", [D, D], f32, isOutput=False)
    iota_ext = nc.declare_dram_parameter("iota", [D, D], b16, isOutput=False)
    idf_ext = nc.declare_dram_parameter("idf", [D, D], f32, isOutput=False)
    out_ext = nc.declare_dram_parameter("out", [NLOC, D], f32, isOutput=True)

    Relu = mybir.ActivationFunctionType.Relu
    Copy = mybir.ActivationFunctionType.Copy
    Ident = mybir.ActivationFunctionType.Identity
    Sqrt = mybir.ActivationFunctionType.Sqrt
    EQ = mybir.AluOpType.is_equal
    ADD = mybir.AluOpType.add
    MUL = mybir.AluOpType.mult

    ngroups = (nchunks + GCH - 1) // GCH

    with tile.TileContext(nc) as tc:
        with (
            tc.tile_pool(name="const", bufs=1) as cpool,
            tc.tile_pool(name="work", bufs=3) as wpool,
            tc.tile_pool(name="gather", bufs=10) as gpool,
            tc.tile_pool(name="small", bufs=4) as spool,
            tc.tile_pool(name="blk", bufs=2) as bpool,
            tc.tile_pool(name="ps2", bufs=2, space="PSUM") as ps2,
            tc.tile_pool(name="ps1", bufs=1, space="PSUM") as ps1,
        ):
            # ---- resident constants -------------------------------------
            gidx = cpool.tile([128, S // 16], i16, tag="gidx")
            nc.sync.dma_start(out=gidx[:], in_=gidx_ext[:])
            dloc = cpool.tile([128, nchunks], f32, tag="dloc")
            nc.sync.dma_start(out=dloc[:], in_=dloc_ext[:])
            cnts = cpool.tile([T, NLOC], b16, tag="cnts")
            nc.sync.dma_start(out=cnts[:], in_=cnts_ext[:])
            xloc = cpool.tile([D, NLOC], b16, tag="xloc")
            nc.sync.dma_start(out=xloc[:], in_=xloc_ext[:])
            wts = cpool.tile([D, T * D], b16, tag="wts")
            nc.sync.dma_start(out=wts[:], in_=wts_ext[:])
            we1 = cpool.tile([D, D], b16, tag="we1")
            nc.sync.dma_start(out=we1[:], in_=we1_ext[:])
            we2 = cpool.tile([D, D], b16, tag="we2")
            nc.sync.dma_start(out=we2[:], in_=we2_ext[:])
            wself = cpool.tile([D, D], b16, tag="wself")
            nc.sync.dma_start(out=wself[:], in_=wself_ext[:])
            bp = cpool.tile([T, D], b16, tag="bp")
            nc.sync.dma_start(out=bp[:], in_=bp_ext[:])
            be1 = cpool.tile([1, D], b16, tag="be1")
            nc.sync.dma_start(out=be1[:], in_=be1_ext[:])
            bself = cpool.tile([D, 1], f32, tag="bself")
            nc.sync.dma_start(out=bself[:], in_=bself_ext[:])
            lng = cpool.tile([D, D], f32, tag="lng")
            nc.sync.dma_start(out=lng[:], in_=lng_ext[:])
            lnb = cpool.tile([D, D], f32, tag="lnb")
            nc.sync.dma_start(out=lnb[:], in_=lnb_ext[:])
            iota = cpool.tile([D, D], b16, tag="iota")
            nc.sync.dma_start(out=iota[:], in_=iota_ext[:])
            idf = cpool.tile([D, D], f32, tag="idf")
            nc.sync.dma_start(out=idf[:], in_=idf_ext[:])
            ones = cpool.tile([1, D], b16, tag="ones")
            nc.gpsimd.memset(ones[:], 1.0)
            epst = cpool.tile([128, 1], f32, tag="epst")
            nc.gpsimd.memset(epst[:], EPS)

            gsems = [nc.alloc_semaphore(f"gsem{q}") for q in range(4)]
            gcount = [0, 0, 0, 0]
            xg_tiles = []
            ef_tiles = []

            # ---- gather groups (<=1024 idxs per dma_gather call), spread
            # across the 4 SWDGE queues so all Q7 cpu pairs generate
            # descriptors in parallel -------------------------------------
            for g in range(ngroups):
                q = 0
                c0 = g * GCH
                ncg = min(GCH, nchunks - c0)
                xg = gpool.tile([128, GCH, D], b16, tag="xg")
                nc.gpsimd.dma_gather(
                    xg[:, 0:ncg, :],
                    xt_ext[IDX_BASE:, :],
                    gidx[:, c0 * 8 : (c0 + ncg) * 8],
                    ncg * 128,
                    ncg * 128,
                    D,
                    prepare_only=True,
                    sem=gsems[q],
                    queue_num=q,
                )
                nc.gpsimd.trigger_dma(count=None, queue_num=q)
                gcount[q] += 16
                with tc.tile_critical():
                    nc.vector.tensor_copy(
                        xg[0:1, 0:1, 0:1], xg[0:1, 0:1, 0:1]
                    )._wait_ge(gsems[q], gcount[q])
                xg_tiles.append(xg)
            for g in range((nchunks + EFG - 1) // EFG):
                c0 = g * EFG
                ncg = min(EFG, nchunks - c0)
                ef = wpool.tile([D, EFG * 128], b16, tag="ef")
                nc.sync.dma_start(
                    out=ef[:, 0 : ncg * 128],
                    in_=eft_ext[:, c0 * 128 : (c0 + ncg) * 128],
                )
                ef_tiles.append(ef)

            # ---- chunk + block loop -------------------------------------
            psS = psP = None
            for ci, ch in enumerate(plan):
                xg, lc = xg_tiles[ci // GCH], ci % GCH
                ef, le = ef_tiles[ci // EFG], ci % EFG
                b, t = ch["b"], ch["t"]
                nb = min(128, NLOC - b * 128)
                if ch["p_start"]:
                    psS = ps2.tile([128, T * D], mybir.dt.float32, tag="S")
                    psP = ps2.tile([128, D], mybir.dt.float32, tag="P")

                # H1 = relu(ef @ We1 + be1)
                h1p = ps2.tile([128, D], mybir.dt.float32, tag="h1p")
                nc.tensor.matmul(
                    h1p[:], ef[:, le * 128 : (le + 1) * 128], we1[:],
                    start=True, stop=False,
                )
                nc.tensor.matmul(h1p[:], ones[:], be1[:], start=False, stop=True)
                h1 = spool.tile([128, D], b16, tag="h1")
                nc.scalar.activation(h1[:], h1p[:], Relu)

                # onehot[e, n] = (iota == d[e])
                oh = spool.tile([128, D], b16, tag="oh")
                nc.vector.tensor_scalar(
                    oh[:], iota[:], dloc[:, ci : ci + 1], None, EQ
                )

                # segment matmuls: S_t^T += x_col^T @ oh ; P^T += H1^T @ oh
                nc.tensor.matmul(
                    psS[:, t * D : (t + 1) * D], xg[:, lc, :], oh[:],
                    start=ch["s_start"], stop=ch["s_stop"],
                )
                nc.tensor.matmul(
                    psP[:], h1[:], oh[:], start=ch["p_start"], stop=ch["p_stop"]
                )

                if not ch["last_of_block"]:
                    continue

                # ---- block epilogue -------------------------------------
                s_sb = bpool.tile([128, T * D], b16, tag="s_sb")
                nc.vector.tensor_copy(s_sb[:], psS[:])
                p_sb = bpool.tile([128, D], b16, tag="p_sb")
                nc.vector.tensor_copy(p_sb[:], psP[:])

                op = ps1.tile([128, D], mybir.dt.float32, tag="op")
                n0 = b * 128
                for tt in range(T):
                    nc.tensor.matmul(
                        op[:, 0:nb],
                        wts[:, tt * D : (tt + 1) * D],
                        s_sb[:, tt * D : tt * D + nb],
                        start=(tt == 0), stop=False,
                    )
                nc.tensor.matmul(op[:, 0:nb], we2[:], p_sb[:, 0:nb],
                                 start=False, stop=False)
                nc.tensor.matmul(op[:, 0:nb], wself[:], xloc[:, n0 : n0 + nb],
                                 start=False, stop=False)
                nc.tensor.matmul(op[:, 0:nb], bp[:], cnts[:, n0 : n0 + nb],
                                 start=False, stop=True)

                gn = bpool.tile([128, D], mybir.dt.float32, tag="gn")
                nc.scalar.activation(gn[:, 0:nb], op[:, 0:nb], Ident,
                                     bias=bself[:, 0:1])

                tp = ps1.tile([128, D], mybir.dt.float32, tag="tp")
                nc.tensor.transpose(tp[0:nb, :], gn[:, 0:nb], idf[:])

                # LayerNorm over free dim + relu
                mu = spool.tile([128, 1], mybir.dt.float32, tag="mu")
                nc.vector.reduce_sum(mu[0:nb], tp[0:nb, :],
                                     axis=mybir.AxisListType.X)
                nc.scalar.activation(mu[0:nb], mu[0:nb], Copy, scale=-1.0 / D)
                xc = bpool.tile([128, D], mybir.dt.float32, tag="xc")
                nc.vector.tensor_scalar(xc[0:nb, :], tp[0:nb, :], mu[0:nb],
                                        None, ADD)
                sq = bpool.tile([128, D], mybir.dt.float32, tag="sq")
                nc.vector.tensor_tensor(sq[0:nb, :], xc[0:nb, :], xc[0:nb, :],
                                        MUL)
                var = spool.tile([128, 1], mybir.dt.float32, tag="var")
                nc.vector.reduce_sum(var[0:nb], sq[0:nb, :],
                                     axis=mybir.AxisListType.X)
                std = spool.tile([128, 1], mybir.dt.float32, tag="std")
                nc.scalar.activation(std[0:nb], var[0:nb], Sqrt,
                                     bias=epst[0:nb], scale=1.0 / D)
                rstd = spool.tile([128, 1], mybir.dt.float32, tag="rstd")
                nc.vector.reciprocal(rstd[0:nb], std[0:nb])
                y = bpool.tile([128, D], mybir.dt.float32, tag="y")
                nc.vector.tensor_scalar(y[0:nb, :], xc[0:nb, :], rstd[0:nb],
                                        None, MUL)
                if not ln_trivial:
                    nc.vector.tensor_tensor(y[0:nb, :], y[0:nb, :],
                                            lng[0:nb, :], MUL)
                    nc.vector.tensor_tensor(y[0:nb, :], y[0:nb, :],
                                            lnb[0:nb, :], ADD)
                outf = bpool.tile([128, D], mybir.dt.float32, tag="outf")
                nc.scalar.activation(outf[0:nb, :], y[0:nb, :], Relu)
                nc.sync.dma_start(out=out_ext[n0 : n0 + nb, :],
                                  in_=outf[0:nb, :])

    nc.finalize()
    return nc


@contextlib.contextmanager
def _maybe_profile():
    if not _PROFILE_DIR:
        yield
        return
    lib = ctypes.CDLL("/opt/axon/libaxon_pjrt.so")
    lib.axon_start_nrt_profile.argtypes = [ctypes.POINTER(ctypes.c_int64),
                                           ctypes.c_size_t]
    lib.axon_start_nrt_profile.restype = ctypes.c_int64
    lib.axon_stop_nrt_profile.argtypes = [ctypes.c_char_p]
    lib.axon_stop_nrt_profile.restype = ctypes.c_int64
    import jax
    jax.devices()
    ids = (ctypes.c_int64 * NC)(*range(NC))
    rc = lib.axon_start_nrt_profile(ids, NC)
    if rc != 0:
        raise RuntimeError(f"axon_start_nrt_profile rc={rc}")
    try:
        yield
    finally:
        os.makedirs(_PROFILE_DIR, exist_ok=True)
        n = lib.axon_stop_nrt_profile(_PROFILE_DIR.encode())
        print(f"profile: {n} file(s) in {_PROFILE_DIR}", file=sys.stderr)


def kernel(x, edge_index, edge_type, edge_features,
           W_types, b_types, W_self, b_self,
           W_e1, b_e1, W_e2, b_e2, ln_g, ln_b):
    x = np.asarray(x, np.float32)
    edge_index = np.asarray(edge_index)
    edge_type = np.asarray(edge_type)
    edge_features = np.asarray(edge_features, np.float32)

    row = edge_index[0].astype(np.int64)
    col = edge_index[1].astype(np.int64)
    ty = np.asarray(edge_type).astype(np.int64)
    core = row // NLOC
    r_loc = row % NLOC
    blk = r_loc // 128
    dl = r_loc % 128
    cell = blk * T + ty

    cnt = np.zeros((NC, NCELL), np.int64)
    np.add.at(cnt, (core, cell), 1)
    plan, nchunks, cell_chunk_start = _plan(cnt.max(axis=0))
    S = nchunks * 128
    cell_slot_start = cell_chunk_start * 128

    ln_trivial = bool(np.all(np.asarray(ln_g) == 1.0)
                      and np.all(np.asarray(ln_b) == 0.0))
    key = (nchunks, tuple(cell_chunk_start.tolist()), ln_trivial)
    if key not in _CACHE:
        _CACHE[key] = _build(key, plan, nchunks, ln_trivial)
    nc = _CACHE[key]

    # shared constants
    xt = x.astype(bf16)
    wts = np.ascontiguousarray(
        np.asarray(W_types, np.float32).transpose(1, 0, 2).reshape(D, T * D)
    ).astype(bf16)
    we1 = np.asarray(W_e1, np.float32).astype(bf16)
    we2 = np.asarray(W_e2, np.float32).astype(bf16)
    wself = np.asarray(W_self, np.float32).astype(bf16)
    bpv = (np.asarray(b_types, np.float32)
           + np.asarray(b_e2, np.float32)[None, :]).astype(bf16)
    be1 = np.asarray(b_e1, np.float32).reshape(1, D).astype(bf16)
    bselfv = np.asarray(b_self, np.float32).reshape(D, 1)
    lngv = np.tile(np.asarray(ln_g, np.float32), (D, 1))
    lnbv = np.tile(np.asarray(ln_b, np.float32), (D, 1))
    iotav = np.tile(np.arange(D, dtype=np.float32), (D, 1)).astype(bf16)
    idfv = np.eye(D, dtype=np.float32)

    in_maps = []
    for c in range(NC):
        m = core == c
        e_ids = np.nonzero(m)[0]
        order = np.argsort(cell[e_ids], kind="stable")
        e_s = e_ids[order]
        cs = cell[e_s]
        cc = np.bincount(cs, minlength=NCELL)
        grp_start = np.concatenate([[0], np.cumsum(cc)])[:-1]
        within = np.arange(len(e_s)) - grp_start[cs]
        slots = cell_slot_start[cs] + within

        gidx = np.full(S, 1, np.int16)
        gidx[slots] = (col[e_s] - IDX_BASE).astype(np.int16)

        dloc = np.full(S, 255.0, np.float32)
        dloc[slots] = dl[e_s]

        eft = np.zeros((D, S), bf16)
        eft[:, slots] = edge_features[e_s].astype(bf16).T

        # the gather ucode trims trailing negative idxs, desyncing descriptor
        # accounting -> ensure each call's final index is non-negative by
        # swapping within the call's last chunk (order inside a chunk is free)
        for g0 in range(0, S, GCH * 128):
            last = min(g0 + GCH * 128, S) - 1
            if gidx[last] < 0:
                cand = np.nonzero(gidx[last - 127 : last] >= 0)[0]
                assert cand.size, "no non-negative gather idx in final chunk"
                j = last - 127 + cand[-1]
                for arr in (gidx, dloc):
                    arr[[j, last]] = arr[[last, j]]
                eft[:, [j, last]] = eft[:, [last, j]]

        gidx_w = np.tile(gidx.reshape(S // 16, 16).T, (8, 1))
        dloc_t = np.ascontiguousarray(dloc.reshape(nchunks, 128).T)

        cnts4 = np.zeros((T, NLOC), np.float32)
        np.add.at(cnts4, (ty[m], r_loc[m]), 1.0)

        in_maps.append({
            "xt": xt,
            "eft": eft,
            "gidx": gidx_w,
            "dloc": dloc_t,
            "cnts": cnts4.astype(bf16),
            "xloc": np.ascontiguousarray(
                x[c * NLOC:(c + 1) * NLOC].T).astype(bf16),
            "wts": wts, "we1": we1, "we2": we2, "wself": wself,
            "bp": bpv, "be1": be1, "bself": bselfv,
            "lng": lngv, "lnb": lnbv, "iota": iotav, "idf": idfv,
        })

    from concourse import bass2jax
    with _maybe_profile():
        results = bass2jax.run_bass_via_pjrt(nc, in_maps, n_cores=NC)
    return np.concatenate([np.asarray(results[c]["out"], np.float32)
                           for c in range(NC)], axis=0)

